# revision 32
# baseline (speedup 1.0000x reference)
"""Trainium2 Bass kernel for nn_BasicBlockBit (ResNet BasicBlock, ternary convs).

Math (per reference):
    out = silu(bn2(conv3x3(silu(bn1(conv3x3(x, q(w1)) + b1)), q(w2)) + b2) + x)
with q() = BitNet ternary quantization (per-tensor median scale).

Strategy:
  - Pure data parallelism: batch 32 -> 4 images per core across 8 cores.
  - fp8-e4m3 DoubleRow matmuls (2 taps per PE instruction). conv1: 4 tap
    pairs + a 113-wide DoubleRow correction over taps ((0,-1),(0,1)) fed
    with dx8 = e4m3(16*(x - e4m3(x))) and weights ternary/16 (corrects the
    first row of each 4-row block) + 1 exact fp16 center tap. conv2: 4 tap
    pairs fp8 + 1 exact fp16 center tap (from mid16). Same numerics as the
    previous kernel: measured end-to-end rel err 1.980e-2 < 2e-2.
    The dx8 stream is shipped compacted (only the 115-elem span per block
    the correction actually reads: 0.41 MB/image instead of 1.65).
  - Two-block interleaving: accumulation chains for two adjacent 4-row
    blocks are interleaved on the PE (ping-pong between two PSUM tiles).
    Measured on hw: back-to-back dependent DR matmuls into one PSUM tile
    cost 229 ns each (452 cols); interleaved across two tiles they hit the
    191 ns floor (1 col/cycle @ 2.37 GHz).
  - Separator layout: image rows are stored with stride 113 (112 pixels +
    one zero column) plus a zero pad row above/below. Every 3x3 tap window
    of a 4-row block is then one contiguous 452-element slab whose
    out-of-image reads land on zeros == exact zero padding. DR pairs use a
    custom overlapping AP [128, 2(step=tap delta), 452].
  - Epilogues: conv1: ACT Silu(ps*scale1+bias1) -> fp16 mid, DVE copy to
    fp8 mid (vector, not gpsimd: gpsimd CAST measured 2005 ns vs ~450 DVE).
    conv2: DVE stt (ps*scale2 + x16), ACT Silu(+bias2) -> fp16 staging ->
    DMA out (host upcasts to f32).
"""

import sys

import numpy as np
import ml_dtypes

try:  # concourse normally resolves via the environment's sitecustomize
    import concourse  # noqa: F401
except ImportError:  # pragma: no cover
    sys.path.insert(0, "/opt/trn_rl_repo")

C = 128
H = W = 112
SW = 113                 # row stride: 112 pixels + 1 zero separator
LROWS = 114              # pad row + 112 rows + pad row
L = SW * LROWS + 14      # 12896; window reads reach index 12883
NPC = 4                  # images per core
NCORES = 8
RB = 4                   # image rows per PSUM tile
NPIX = RB * SW           # 452 psum columns (448 valid)
NOUT = RB * W            # 448
NBLK = H // RB           # 28
BN_EPS = 1e-5

# taps as (dy, dx) in {-1,0,1}; offset in separator layout = 113*dy + dx
def _toff(dy, dx):
    return SW * dy + dx

# 8 non-center taps as 4 DR pairs; center handled specially per conv
PAIRS = [((-1, -1), (-1, 1)), ((0, -1), (0, 1)), ((1, -1), (1, 1)),
         ((-1, 0), (1, 0))]
CORRW = SW   # first row of each block gets the dx8 correction
CORRW2 = SW + 2  # compact dx8 row span: cols base-1 .. base+113

_CACHE = {}


def _build_nc():
    import concourse.mybir as mybir
    from concourse import bacc, bass
    from concourse.tile import TileContext

    f32 = mybir.dt.float32
    f16 = mybir.dt.float16
    f8 = mybir.dt.float8e4
    bf16 = mybir.dt.bfloat16
    DR = mybir.MatmulPerfMode.DoubleRow
    Silu = mybir.ActivationFunctionType.Silu
    mult = mybir.AluOpType.mult
    add = mybir.AluOpType.add

    nc = bacc.Bacc(trn_type="TRN2", target_bir_lowering=False, debug=False)

    x16in = nc.dram_tensor("x16in", [NPC, C, L], f16, kind="ExternalInput")
    x8in = nc.dram_tensor("x8in", [NPC, C, L], f8, kind="ExternalInput")
    # dx8 = e4m3(16*(x-x8)), compacted: only the first padded row of each
    # 4-row block (cols base-1..base+113) is read by the correction pass
    dx8cin = nc.dram_tensor("dx8cin", [NPC, C, NBLK * CORRW2], f8,
                            kind="ExternalInput")
    wt1q_d = nc.dram_tensor("wt1q", [C, len(PAIRS), 2, C], f8, kind="ExternalInput")
    wt1c_d = nc.dram_tensor("wt1c", [C, 2, C], f8, kind="ExternalInput")
    wt1e_d = nc.dram_tensor("wt1e", [C, C], f16, kind="ExternalInput")
    wt2e_d = nc.dram_tensor("wt2e", [C, C], f16, kind="ExternalInput")
    wt2q_d = nc.dram_tensor("wt2q", [C, len(PAIRS), 2, C], f8, kind="ExternalInput")
    # columns: scale1, bias1, scale2, bias2
    vecs = nc.dram_tensor("vecs", [C, 4], f32, kind="ExternalInput")
    out = nc.dram_tensor("out", [NPC, C, H * W], f16, kind="ExternalOutput")

    with TileContext(nc) as tc:
        with (
            tc.tile_pool(name="consts", bufs=1) as consts,
            tc.tile_pool(name="x16pool", bufs=2) as x16pool,
            tc.tile_pool(name="x8pool", bufs=2) as x8pool,
            tc.tile_pool(name="dx8pool", bufs=2) as dx8pool,
            tc.tile_pool(name="mid16pool", bufs=1) as mid16pool,
            tc.tile_pool(name="mid8pool", bufs=1) as mid8pool,
            tc.tile_pool(name="pspool", bufs=8, space="PSUM") as pspool,
            tc.tile_pool(name="otpool", bufs=2) as otpool,
            tc.tile_pool(name="otlpool", bufs=4) as otlpool,
            tc.tile_pool(name="stpool", bufs=2) as stpool,
            tc.tile_pool(name="stlpool", bufs=4) as stlpool,
        ):
            # First image's leading rows + conv1 weights go first so the PE
            # can start as early as possible.
            w1q_sb = consts.tile([C, len(PAIRS), 2, C], f8, name="w1q_sb")
            w1c_sb = consts.tile([C, 2, C], f8, name="w1c_sb")
            w1e_sb = consts.tile([C, C], f16, name="w1e_sb")
            vecs_sb = consts.tile([C, 4], f32, name="vecs_sb")
            w2e_sb = consts.tile([C, C], f16, name="w2e_sb")
            w2q_sb = consts.tile([C, len(PAIRS), 2, C], f8, name="w2q_sb")
            x0_16 = x16pool.tile([C, L], f16, name="x16", tag="x16")
            x0_8 = x8pool.tile([C, L], f8, name="x8", tag="x8")
            x0_dc = dx8pool.tile([C, NBLK * CORRW2], f8, name="dx8c", tag="dx8c")

            # image-0 input: three streams on three DMA queues (wire-bound
            # start: finer chunks + parallel descriptor injection). x8 rows
            # lead on their queue: the first 4 DR passes of each block only
            # need x8; x16/dx8c arrive before passes 5-6.
            def img0_chunk(r0, r1):
                a, b = r0 * SW, (L if r1 >= LROWS else r1 * SW)
                nc.scalar.dma_start(x0_8[:, a:b], x8in.ap()[0, :, a:b])
                nc.sync.dma_start(x0_16[:, a:b], x16in.ap()[0, :, a:b])

            nc.sync.dma_start(w1q_sb[:, :, :, :], wt1q_d.ap())
            nc.gpsimd.dma_start(w1c_sb[:, :, :], wt1c_d.ap())
            nc.gpsimd.dma_start(w1e_sb[:, :], wt1e_d.ap())
            img0_chunk(0, 4)
            nc.gpsimd.dma_start(x0_dc[:, 0 : 2 * CORRW2],
                                dx8cin.ap()[0, :, 0 : 2 * CORRW2])
            img0_chunk(4, 8)
            nc.gpsimd.dma_start(vecs_sb[:, :], vecs.ap())
            img0_chunk(8, 12)
            img0_chunk(12, 16)
            nc.gpsimd.dma_start(
                x0_dc[:, 2 * CORRW2 : NBLK * CORRW2],
                dx8cin.ap()[0, :, 2 * CORRW2 : NBLK * CORRW2],
            )
            img0_chunk(16, 24)
            img0_chunk(24, 32)
            nc.gpsimd.dma_start(w2q_sb[:, :, :, :], wt2q_d.ap())
            nc.gpsimd.dma_start(w2e_sb[:, :], wt2e_d.ap())
            img0_chunk(32, 40)
            img0_chunk(40, 48)
            img0_chunk(48, 64)
            img0_chunk(64, 86)
            img0_chunk(86, LROWS)
            scale1 = vecs_sb[:, 0:1]
            bias1 = vecs_sb[:, 1:2]
            scale2 = vecs_sb[:, 2:3]
            bias2 = vecs_sb[:, 3:4]

            # Warm the PE HAM clock gate while the first DMAs are in flight
            # (cold PE runs at 1.2 GHz; ~3.4us of activity un-throttles it).
            warm_sb = consts.tile([C, 452], bf16, name="warm_sb")
            nc.vector.memset(warm_sb[:, :], 0.0)
            # 6 x ~375ns cold matmuls: ends near input-data arrival; the
            # first conv matmuls finish the ~3.4us HAM activity window at
            # the mid pstate while doing useful work.
            warm_ps = pspool.tile([C, NPIX], f32, name="warm_ps", tag="ps")
            for _ in range(6):
                nc.tensor.matmul(
                    warm_ps[:, :], warm_sb[:, 0:128], warm_sb[:, :],
                    start=True, stop=True,
                )

            last_ps = None
            for img in range(NPC):
                if img == 0:
                    x16 = x0_16
                    x8 = x0_8
                    dx8c = x0_dc
                else:
                    x16 = x16pool.tile([C, L], f16, name="x16", tag="x16")
                    x8 = x8pool.tile([C, L], f8, name="x8", tag="x8")
                    dx8c = dx8pool.tile([C, NBLK * CORRW2], f8, name="dx8c",
                                        tag="dx8c")
                    nc.sync.dma_start(dx8c[:, :], dx8cin.ap()[img, :, :])
                    for r0, r1 in ((0, 57 * SW), (57 * SW, L)):
                        nc.sync.dma_start(x16[:, r0:r1], x16in.ap()[img, :, r0:r1])
                        nc.sync.dma_start(x8[:, r0:r1], x8in.ap()[img, :, r0:r1])

                def x8_pair(base, delta, width=NPIX):
                    # overlapping [128, 2, width] moving AP for a DR tap pair
                    return bass.AP(x8.tensor, x8.offset + base,
                                   [[L, C], [delta, 2], [1, width]])

                def dx8c_pair(blk):
                    # corr pair over taps ((0,-1),(0,1)) in the compact dx8
                    # row for block blk (row starts at col base-1)
                    return bass.AP(dx8c.tensor, dx8c.offset + blk * CORRW2,
                                   [[NBLK * CORRW2, C], [2, 2], [1, CORRW]])

                def m8_pair(base, delta, width=NPIX):
                    return bass.AP(mid8.tensor, mid8.offset + base,
                                   [[L, C], [delta, 2], [1, width]])

                mid16 = mid16pool.tile([C, L], f16, name="mid16", tag="mid16")
                mid8 = mid8pool.tile([C, L], f8, name="mid8", tag="mid8")
                # zero borders: top pad row (incl row 0's separator), bottom
                # pad row + tail, interior separators (one per row)
                for m in (mid16, mid8):
                    m4 = m[:, 0 : SW * LROWS].rearrange("p (h w) -> p h w", h=LROWS)
                    nc.vector.memset(m[:, 0 : SW + 1], 0.0)
                    nc.vector.memset(m[:, SW * (LROWS - 1) : L], 0.0)
                    nc.vector.memset(m4[:, 2 : LROWS - 1, 0:1], 0.0)

                x4 = x16[:, 0 : SW * LROWS].rearrange("p (h w) -> p h w", h=LROWS)
                m16_4 = mid16[:, 0 : SW * LROWS].rearrange("p (h w) -> p h w", h=LROWS)

                # ---- conv1 + bn1 + silu -> mid16 (fp16) and mid8 (fp8) ----
                # two-block interleaved accumulation chains
                for bp in range(NBLK // 2):
                    blks = (2 * bp, 2 * bp + 1)
                    pss = []
                    bases = []
                    for b in blks:
                        pss.append(pspool.tile([C, NPIX], f32, name="ps", tag="ps"))
                        bases.append(SW * (b * RB + 1) + 1)
                    for p, (ta, tb) in enumerate(PAIRS):
                        d = _toff(*tb) - _toff(*ta)
                        for ps, base in zip(pss, bases):
                            nc.tensor.matmul(
                                ps[:, :], w1q_sb[:, p, :, :],
                                x8_pair(base + _toff(*ta), d),
                                start=(p == 0), stop=False, perf_mode=DR,
                            )
                    for ps, b in zip(pss, blks):
                        nc.tensor.matmul(
                            ps[:, 0:CORRW], w1c_sb[:, :, :], dx8c_pair(b),
                            start=False, stop=False, perf_mode=DR,
                        )
                    for ps, base in zip(pss, bases):
                        o = base + _toff(0, 0)
                        nc.tensor.matmul(
                            ps[:, :], w1e_sb[:, :], x16[:, o : o + NPIX],
                            start=False, stop=True,
                        )
                    for ps, b, base in zip(pss, blks, bases):
                        h0 = b * RB
                        ps3 = ps.rearrange("p (h w) -> p h w", h=RB)
                        nc.scalar.activation(
                            m16_4[:, h0 + 1 : h0 + 1 + RB, 1 : 1 + W],
                            ps3[:, :, 0:W],
                            Silu,
                            bias=bias1,
                            scale=scale1,
                        )
                        # contiguous 452-span copy (separators zero in both)
                        nc.vector.tensor_copy(
                            mid8[:, base - 1 : base - 1 + NPIX],
                            mid16[:, base - 1 : base - 1 + NPIX],
                        )

                # ---- conv2 + bn2 + residual + silu -> out ----
                GS = 4
                st = None
                ot = None
                for bp in range(NBLK // 2):
                    blks = (2 * bp, 2 * bp + 1)
                    pss = []
                    bases = []
                    for b in blks:
                        pss.append(pspool.tile([C, NPIX], f32, name="ps", tag="ps"))
                        bases.append(SW * (b * RB + 1) + 1)
                    for p, (ta, tb) in enumerate(PAIRS):
                        d = _toff(*tb) - _toff(*ta)
                        for ps, base in zip(pss, bases):
                            nc.tensor.matmul(
                                ps[:, :], w2q_sb[:, p, :, :],
                                m8_pair(base + _toff(*ta), d),
                                start=(p == 0), stop=False, perf_mode=DR,
                            )
                    for ps, base in zip(pss, bases):
                        nc.tensor.matmul(
                            ps[:, :], w2e_sb[:, :], mid16[:, base : base + NPIX],
                            start=False, stop=True,
                        )
                    for ps, blk in zip(pss, blks):
                        h0 = blk * RB
                        ps3 = ps.rearrange("p (h w) -> p h w", h=RB)
                        xw = x4[:, h0 + 1 : h0 + 1 + RB, 1 : 1 + W]
                        last_group = img == NPC - 1 and blk >= NBLK - GS
                        if last_group:
                            # per-block epilogue+store at the very end
                            # shortens the tail after the final matmul
                            otl = otlpool.tile([C, NOUT], f16, name="otl", tag="otl")
                            nc.vector.scalar_tensor_tensor(
                                otl.rearrange("p (h w) -> p h w", h=RB),
                                ps3[:, :, 0:W], scale2, xw, mult, add,
                            )
                            stl = stlpool.tile([C, NOUT], f16, name="stl", tag="stl")
                            nc.scalar.activation(stl[:, :], otl[:, :], Silu, bias=bias2)
                            nc.sync.dma_start(
                                out.ap()[img, :, h0 * W : (h0 + RB) * W], stl[:, :]
                            )
                            if blk == NBLK - 2:
                                # drain target: first tile of the last pair,
                                # whose stt retires earliest
                                last_ps = ps
                            continue
                        g = blk % GS
                        if g == 0:
                            ot = otpool.tile([C, GS * NOUT], f16, name="ot", tag="ot")
                            st = stpool.tile([C, GS * NOUT], f16, name="st", tag="st")
                        # fused: ot = ps*scale2 + x; silu bias folds into ACT
                        nc.vector.scalar_tensor_tensor(
                            ot[:, g * NOUT : (g + 1) * NOUT].rearrange(
                                "p (h w) -> p h w", h=RB
                            ),
                            ps3[:, :, 0:W], scale2, xw, mult, add,
                        )
                        if g == GS - 1:
                            nc.scalar.activation(st[:, :], ot[:, :], Silu, bias=bias2)
                            nc.sync.dma_start(
                                out.ap()[img, :, (h0 - (GS - 1) * RB) * W : (h0 + RB) * W],
                                st[:, :],
                            )

            # trailing no-consumer matmul: the TileContext-exit DRAIN on the
            # PE queue otherwise swallows the last block's completion
            # semaphore flush. Writes over the last conv2 psum tile (WAR on
            # its stt read) so it fires right after the last epilogue read
            # instead of waiting for a free pool slot.
            nc.tensor.matmul(
                last_ps[:, 0:64], w2e_sb[:, :], w2e_sb[:, 0:64],
                start=True, stop=True,
            )

    nc.compile()
    return nc


def _quantize_ternary(w):
    """BitNet ternary quantization, matching the jax reference in fp32."""
    w = np.asarray(w, np.float32)
    scale = np.float32(max(np.float32(np.median(np.abs(w))), np.float32(1e-8)))
    tern = np.clip(np.round(w / scale), -1.0, 1.0).astype(np.float32)
    return tern, scale


def _pack_pairs(tern, pairs, f8dt):
    # lhsT layout [cin, pair, 2, cout] fp8
    return np.ascontiguousarray(
        np.stack(
            [
                np.stack([tern[:, :, ta[0] + 1, ta[1] + 1].T,
                          tern[:, :, tb[0] + 1, tb[1] + 1].T], axis=1)
                for (ta, tb) in pairs
            ],
            axis=1,
        ).astype(f8dt)
    )


def _host_prep(x, w1, b1, g1, be1, m1, v1, w2, b2, g2, be2, m2, v2):
    t1, s1 = _quantize_ternary(w1)
    t2, s2 = _quantize_ternary(w2)
    f8 = ml_dtypes.float8_e4m3
    wt1q = _pack_pairs(t1, PAIRS, f8)
    # corr pair over taps ((0,-1),(0,1)) with weights ternary/16
    tc16 = t1 / 16.0
    wt1c = np.ascontiguousarray(
        np.stack([tc16[:, :, 1, 0].T, tc16[:, :, 1, 2].T], axis=1).astype(f8)
    )
    wt1e = np.ascontiguousarray(t1[:, :, 1, 1].T.astype(np.float16))
    wt2q = _pack_pairs(t2, PAIRS, f8)
    wt2e = np.ascontiguousarray(t2[:, :, 1, 1].T.astype(np.float16))
    inv1 = (g1 / np.sqrt(v1 + BN_EPS)).astype(np.float32)
    inv2 = (g2 / np.sqrt(v2 + BN_EPS)).astype(np.float32)
    scale1 = s1 * inv1
    bias1 = b1 * inv1 + be1 - m1 * inv1
    scale2 = s2 * inv2
    bias2 = b2 * inv2 + be2 - m2 * inv2
    vecs = np.stack([scale1, bias1, scale2, bias2], axis=1).astype(np.float32)

    n = x.shape[0]
    x16 = np.zeros((n, C, L), dtype=np.float16)
    x8 = np.zeros((n, C, L), dtype=f8)
    dx8f = np.zeros((n, C, L), dtype=f8)
    x8v = x.astype(f8)
    dxv = np.clip(16.0 * (x - x8v.astype(np.float32)), -240, 240)
    for arr, val in ((x16, x), (x8, x8v), (dx8f, dxv)):
        a4 = arr[:, :, 0 : SW * LROWS].reshape(n, C, LROWS, SW)
        a4[:, :, 1 : 1 + H, 1 : 1 + W] = val
    # compact dx8: per block, the 115-elem span the corr pair reads
    dx8c = np.zeros((n, C, NBLK * CORRW2), dtype=f8)
    for b in range(NBLK):
        o = SW * (b * RB + 1)
        dx8c[:, :, b * CORRW2 : (b + 1) * CORRW2] = dx8f[:, :, o : o + CORRW2]
    return x16, x8, dx8c, wt1q, wt1c, wt1e, wt2e, wt2q, vecs


def kernel(
    x,
    w1,
    b1,
    bn1_gamma,
    bn1_beta,
    bn1_mean,
    bn1_var,
    w2,
    b2,
    bn2_gamma,
    bn2_beta,
    bn2_mean,
    bn2_var,
    _trace=False,
):
    from concourse.bass_utils import run_bass_kernel_spmd

    x = np.asarray(x, np.float32)
    w1, b1, w2, b2 = (np.asarray(a, np.float32) for a in (w1, b1, w2, b2))
    bn1_gamma, bn1_beta, bn1_mean, bn1_var = (
        np.asarray(a, np.float32) for a in (bn1_gamma, bn1_beta, bn1_mean, bn1_var)
    )
    bn2_gamma, bn2_beta, bn2_mean, bn2_var = (
        np.asarray(a, np.float32) for a in (bn2_gamma, bn2_beta, bn2_mean, bn2_var)
    )

    x16, x8, dx8c, wt1q, wt1c, wt1e, wt2e, wt2q, vecs = _host_prep(
        x, w1, b1, bn1_gamma, bn1_beta, bn1_mean, bn1_var,
        w2, b2, bn2_gamma, bn2_beta, bn2_mean, bn2_var,
    )

    if "nc" not in _CACHE:
        _CACHE["nc"] = _build_nc()
    nc = _CACHE["nc"]

    in_maps = [
        {
            "x16in": np.ascontiguousarray(x16[i * NPC : (i + 1) * NPC]),
            "x8in": np.ascontiguousarray(x8[i * NPC : (i + 1) * NPC]),
            "dx8cin": np.ascontiguousarray(dx8c[i * NPC : (i + 1) * NPC]),
            "wt1q": wt1q,
            "wt1c": wt1c,
            "wt1e": wt1e,
            "wt2e": wt2e,
            "wt2q": wt2q,
            "vecs": vecs,
        }
        for i in range(NCORES)
    ]
    res = run_bass_kernel_spmd(nc, in_maps, core_ids=list(range(NCORES)), trace=_trace)
    outs = [
        res.results[i]["out"].reshape(NPC, C, H, W).astype(np.float32)
        for i in range(NCORES)
    ]
    full = np.concatenate(outs, axis=0)
    if _trace:
        _CACHE["last_results"] = res
    return full


# revision 34
# speedup vs baseline: 1.0094x; 1.0094x over previous
"""Trainium2 Bass kernel for nn_BasicBlockBit (ResNet BasicBlock, ternary convs).

Math (per reference):
    out = silu(bn2(conv3x3(silu(bn1(conv3x3(x, q(w1)) + b1)), q(w2)) + b2) + x)
with q() = BitNet ternary quantization (per-tensor median scale).

Strategy:
  - Pure data parallelism: batch 32 -> 4 images per core across 8 cores.
  - fp8-e4m3 DoubleRow matmuls (2 taps per PE instruction). conv1: 4 tap
    pairs + a 113-wide DoubleRow correction over taps ((0,-1),(0,1)) fed
    with dx8 = e4m3(16*(x - e4m3(x))) and weights ternary/16 (corrects the
    first row of each 4-row block) + 1 exact fp16 center tap. conv2: 4 tap
    pairs fp8 + 1 exact fp16 center tap (from mid16). Same numerics as the
    previous kernel: measured end-to-end rel err 1.980e-2 < 2e-2.
    The dx8 stream is shipped compacted (only the 115-elem span per block
    the correction actually reads: 0.41 MB/image instead of 1.65).
  - Two-block interleaving: accumulation chains for two adjacent 4-row
    blocks are interleaved on the PE (ping-pong between two PSUM tiles).
    Measured on hw: back-to-back dependent DR matmuls into one PSUM tile
    cost 229 ns each (452 cols); interleaved across two tiles they hit the
    191 ns floor (1 col/cycle @ 2.37 GHz).
  - Separator layout: image rows are stored with stride 113 (112 pixels +
    one zero column) plus a zero pad row above/below. Every 3x3 tap window
    of a 4-row block is then one contiguous 452-element slab whose
    out-of-image reads land on zeros == exact zero padding. DR pairs use a
    custom overlapping AP [128, 2(step=tap delta), 452].
  - Epilogues: conv1: ACT Silu(ps*scale1+bias1) -> fp16 mid, DVE copy to
    fp8 mid (vector, not gpsimd: gpsimd CAST measured 2005 ns vs ~450 DVE).
    conv2: DVE stt (ps*scale2 + x16), ACT Silu(+bias2) -> fp16 staging ->
    DMA out (host upcasts to f32).
"""

import sys

import numpy as np
import ml_dtypes

try:  # concourse normally resolves via the environment's sitecustomize
    import concourse  # noqa: F401
except ImportError:  # pragma: no cover
    sys.path.insert(0, "/opt/trn_rl_repo")

C = 128
H = W = 112
SW = 113                 # row stride: 112 pixels + 1 zero separator
LROWS = 114              # pad row + 112 rows + pad row
L = SW * LROWS + 14      # 12896; window reads reach index 12883
NPC = 4                  # images per core
NCORES = 8
RB = 4                   # image rows per PSUM tile
NPIX = RB * SW           # 452 psum columns (448 valid)
NOUT = RB * W            # 448
NBLK = H // RB           # 28
BN_EPS = 1e-5

# taps as (dy, dx) in {-1,0,1}; offset in separator layout = 113*dy + dx
def _toff(dy, dx):
    return SW * dy + dx

# 8 non-center taps as 4 DR pairs; center handled specially per conv
PAIRS = [((-1, -1), (-1, 1)), ((0, -1), (0, 1)), ((1, -1), (1, 1)),
         ((-1, 0), (1, 0))]
CORRW = SW   # first row of each block gets the dx8 correction
CORRW2 = SW + 2  # compact dx8 row span: cols base-1 .. base+113

_CACHE = {}


def _build_nc():
    import concourse.mybir as mybir
    from concourse import bacc, bass
    from concourse.tile import TileContext

    f32 = mybir.dt.float32
    f16 = mybir.dt.float16
    f8 = mybir.dt.float8e4
    bf16 = mybir.dt.bfloat16
    DR = mybir.MatmulPerfMode.DoubleRow
    Silu = mybir.ActivationFunctionType.Silu
    mult = mybir.AluOpType.mult
    add = mybir.AluOpType.add

    nc = bacc.Bacc(trn_type="TRN2", target_bir_lowering=False, debug=False)

    x16in = nc.dram_tensor("x16in", [NPC, C, L], f16, kind="ExternalInput")
    x8in = nc.dram_tensor("x8in", [NPC, C, L], f8, kind="ExternalInput")
    # dx8 = e4m3(16*(x-x8)), compacted: only the first padded row of each
    # 4-row block (cols base-1..base+113) is read by the correction pass
    dx8cin = nc.dram_tensor("dx8cin", [NPC, C, NBLK * CORRW2], f8,
                            kind="ExternalInput")
    wt1q_d = nc.dram_tensor("wt1q", [C, len(PAIRS), 2, C], f8, kind="ExternalInput")
    wt1c_d = nc.dram_tensor("wt1c", [C, 2, C], f8, kind="ExternalInput")
    wt1e_d = nc.dram_tensor("wt1e", [C, C], f16, kind="ExternalInput")
    wt2e_d = nc.dram_tensor("wt2e", [C, C], f16, kind="ExternalInput")
    wt2q_d = nc.dram_tensor("wt2q", [C, len(PAIRS), 2, C], f8, kind="ExternalInput")
    # columns: scale1, bias1, scale2, bias2
    vecs = nc.dram_tensor("vecs", [C, 4], f32, kind="ExternalInput")
    out = nc.dram_tensor("out", [NPC, C, H * W], f16, kind="ExternalOutput")

    with TileContext(nc) as tc:
        with (
            tc.tile_pool(name="consts", bufs=1) as consts,
            tc.tile_pool(name="x16pool", bufs=2) as x16pool,
            tc.tile_pool(name="x8pool", bufs=2) as x8pool,
            tc.tile_pool(name="dx8pool", bufs=2) as dx8pool,
            tc.tile_pool(name="mid16pool", bufs=1) as mid16pool,
            tc.tile_pool(name="mid8pool", bufs=1) as mid8pool,
            tc.tile_pool(name="pspool", bufs=8, space="PSUM") as pspool,
            tc.tile_pool(name="otpool", bufs=2) as otpool,
            tc.tile_pool(name="otlpool", bufs=4) as otlpool,
            tc.tile_pool(name="stpool", bufs=2) as stpool,
            tc.tile_pool(name="stlpool", bufs=4) as stlpool,
        ):
            # First image's leading rows + conv1 weights go first so the PE
            # can start as early as possible.
            w1q_sb = consts.tile([C, len(PAIRS), 2, C], f8, name="w1q_sb")
            w1c_sb = consts.tile([C, 2, C], f8, name="w1c_sb")
            w1e_sb = consts.tile([C, C], f16, name="w1e_sb")
            vecs_sb = consts.tile([C, 4], f32, name="vecs_sb")
            w2e_sb = consts.tile([C, C], f16, name="w2e_sb")
            w2q_sb = consts.tile([C, len(PAIRS), 2, C], f8, name="w2q_sb")
            x0_16 = x16pool.tile([C, L], f16, name="x16", tag="x16")
            x0_8 = x8pool.tile([C, L], f8, name="x8", tag="x8")
            x0_dc = dx8pool.tile([C, NBLK * CORRW2], f8, name="dx8c", tag="dx8c")

            # image-0 input: three streams on three DMA queues (wire-bound
            # start: finer chunks + parallel descriptor injection). x8 rows
            # lead on their queue: the first 4 DR passes of each block only
            # need x8; x16/dx8c arrive before passes 5-6.
            def img0_chunk(r0, r1):
                a, b = r0 * SW, (L if r1 >= LROWS else r1 * SW)
                nc.scalar.dma_start(x0_8[:, a:b], x8in.ap()[0, :, a:b])
                nc.sync.dma_start(x0_16[:, a:b], x16in.ap()[0, :, a:b])

            nc.sync.dma_start(w1q_sb[:, :, :, :], wt1q_d.ap())
            nc.gpsimd.dma_start(w1c_sb[:, :, :], wt1c_d.ap())
            nc.gpsimd.dma_start(w1e_sb[:, :], wt1e_d.ap())
            img0_chunk(0, 8)
            nc.gpsimd.dma_start(x0_dc[:, 0 : 2 * CORRW2],
                                dx8cin.ap()[0, :, 0 : 2 * CORRW2])
            nc.gpsimd.dma_start(vecs_sb[:, :], vecs.ap())
            img0_chunk(8, 16)
            nc.gpsimd.dma_start(
                x0_dc[:, 2 * CORRW2 : NBLK * CORRW2],
                dx8cin.ap()[0, :, 2 * CORRW2 : NBLK * CORRW2],
            )
            img0_chunk(16, 24)
            img0_chunk(24, 32)
            nc.gpsimd.dma_start(w2q_sb[:, :, :, :], wt2q_d.ap())
            nc.gpsimd.dma_start(w2e_sb[:, :], wt2e_d.ap())
            img0_chunk(32, 40)
            img0_chunk(40, 48)
            img0_chunk(48, 64)
            img0_chunk(64, 86)
            img0_chunk(86, LROWS)
            scale1 = vecs_sb[:, 0:1]
            bias1 = vecs_sb[:, 1:2]
            scale2 = vecs_sb[:, 2:3]
            bias2 = vecs_sb[:, 3:4]

            # Warm the PE HAM clock gate while the first DMAs are in flight
            # (cold PE runs at 1.2 GHz; ~3.4us of activity un-throttles it).
            warm_sb = consts.tile([C, 452], bf16, name="warm_sb")
            nc.vector.memset(warm_sb[:, :], 0.0)
            # 9 x ~375ns cold matmuls end right as image-0 data arrives:
            # shorter warm-ups leave a PE idle gap in which the HAM
            # re-throttles, making the first conv blocks run at 1.2 GHz
            # (measured: warm-6 cost ~2.5us of cold conv matmuls).
            warm_ps = pspool.tile([C, NPIX], f32, name="warm_ps", tag="ps")
            for _ in range(9):
                nc.tensor.matmul(
                    warm_ps[:, :], warm_sb[:, 0:128], warm_sb[:, :],
                    start=True, stop=True,
                )

            last_ps = None
            for img in range(NPC):
                if img == 0:
                    x16 = x0_16
                    x8 = x0_8
                    dx8c = x0_dc
                else:
                    x16 = x16pool.tile([C, L], f16, name="x16", tag="x16")
                    x8 = x8pool.tile([C, L], f8, name="x8", tag="x8")
                    dx8c = dx8pool.tile([C, NBLK * CORRW2], f8, name="dx8c",
                                        tag="dx8c")
                    nc.sync.dma_start(dx8c[:, :], dx8cin.ap()[img, :, :])
                    for r0, r1 in ((0, 57 * SW), (57 * SW, L)):
                        nc.sync.dma_start(x16[:, r0:r1], x16in.ap()[img, :, r0:r1])
                        nc.sync.dma_start(x8[:, r0:r1], x8in.ap()[img, :, r0:r1])

                def x8_pair(base, delta, width=NPIX):
                    # overlapping [128, 2, width] moving AP for a DR tap pair
                    return bass.AP(x8.tensor, x8.offset + base,
                                   [[L, C], [delta, 2], [1, width]])

                def dx8c_pair(blk):
                    # corr pair over taps ((0,-1),(0,1)) in the compact dx8
                    # row for block blk (row starts at col base-1)
                    return bass.AP(dx8c.tensor, dx8c.offset + blk * CORRW2,
                                   [[NBLK * CORRW2, C], [2, 2], [1, CORRW]])

                def m8_pair(base, delta, width=NPIX):
                    return bass.AP(mid8.tensor, mid8.offset + base,
                                   [[L, C], [delta, 2], [1, width]])

                mid16 = mid16pool.tile([C, L], f16, name="mid16", tag="mid16")
                mid8 = mid8pool.tile([C, L], f8, name="mid8", tag="mid8")
                # zero borders: top pad row (incl row 0's separator), bottom
                # pad row + tail, interior separators (one per row)
                for m in (mid16, mid8):
                    m4 = m[:, 0 : SW * LROWS].rearrange("p (h w) -> p h w", h=LROWS)
                    nc.vector.memset(m[:, 0 : SW + 1], 0.0)
                    nc.vector.memset(m[:, SW * (LROWS - 1) : L], 0.0)
                    nc.vector.memset(m4[:, 2 : LROWS - 1, 0:1], 0.0)

                x4 = x16[:, 0 : SW * LROWS].rearrange("p (h w) -> p h w", h=LROWS)
                m16_4 = mid16[:, 0 : SW * LROWS].rearrange("p (h w) -> p h w", h=LROWS)

                # ---- conv1 + bn1 + silu -> mid16 (fp16) and mid8 (fp8) ----
                # two-block interleaved accumulation chains
                for bp in range(NBLK // 2):
                    blks = (2 * bp, 2 * bp + 1)
                    pss = []
                    bases = []
                    for b in blks:
                        pss.append(pspool.tile([C, NPIX], f32, name="ps", tag="ps"))
                        bases.append(SW * (b * RB + 1) + 1)
                    for p, (ta, tb) in enumerate(PAIRS):
                        d = _toff(*tb) - _toff(*ta)
                        for ps, base in zip(pss, bases):
                            nc.tensor.matmul(
                                ps[:, :], w1q_sb[:, p, :, :],
                                x8_pair(base + _toff(*ta), d),
                                start=(p == 0), stop=False, perf_mode=DR,
                            )
                    for ps, b in zip(pss, blks):
                        nc.tensor.matmul(
                            ps[:, 0:CORRW], w1c_sb[:, :, :], dx8c_pair(b),
                            start=False, stop=False, perf_mode=DR,
                        )
                    for ps, base in zip(pss, bases):
                        o = base + _toff(0, 0)
                        nc.tensor.matmul(
                            ps[:, :], w1e_sb[:, :], x16[:, o : o + NPIX],
                            start=False, stop=True,
                        )
                    for ps, b, base in zip(pss, blks, bases):
                        h0 = b * RB
                        ps3 = ps.rearrange("p (h w) -> p h w", h=RB)
                        nc.scalar.activation(
                            m16_4[:, h0 + 1 : h0 + 1 + RB, 1 : 1 + W],
                            ps3[:, :, 0:W],
                            Silu,
                            bias=bias1,
                            scale=scale1,
                        )
                        # contiguous 452-span copy (separators zero in both)
                        nc.vector.tensor_copy(
                            mid8[:, base - 1 : base - 1 + NPIX],
                            mid16[:, base - 1 : base - 1 + NPIX],
                        )

                # ---- conv2 + bn2 + residual + silu -> out ----
                GS = 4
                st = None
                ot = None
                for bp in range(NBLK // 2):
                    blks = (2 * bp, 2 * bp + 1)
                    pss = []
                    bases = []
                    for b in blks:
                        pss.append(pspool.tile([C, NPIX], f32, name="ps", tag="ps"))
                        bases.append(SW * (b * RB + 1) + 1)
                    for p, (ta, tb) in enumerate(PAIRS):
                        d = _toff(*tb) - _toff(*ta)
                        for ps, base in zip(pss, bases):
                            nc.tensor.matmul(
                                ps[:, :], w2q_sb[:, p, :, :],
                                m8_pair(base + _toff(*ta), d),
                                start=(p == 0), stop=False, perf_mode=DR,
                            )
                    for ps, base in zip(pss, bases):
                        nc.tensor.matmul(
                            ps[:, :], w2e_sb[:, :], mid16[:, base : base + NPIX],
                            start=False, stop=True,
                        )
                    for ps, blk in zip(pss, blks):
                        h0 = blk * RB
                        ps3 = ps.rearrange("p (h w) -> p h w", h=RB)
                        xw = x4[:, h0 + 1 : h0 + 1 + RB, 1 : 1 + W]
                        last_group = img == NPC - 1 and blk >= NBLK - GS
                        if last_group:
                            # per-block epilogue+store at the very end
                            # shortens the tail after the final matmul
                            otl = otlpool.tile([C, NOUT], f16, name="otl", tag="otl")
                            nc.vector.scalar_tensor_tensor(
                                otl.rearrange("p (h w) -> p h w", h=RB),
                                ps3[:, :, 0:W], scale2, xw, mult, add,
                            )
                            stl = stlpool.tile([C, NOUT], f16, name="stl", tag="stl")
                            nc.scalar.activation(stl[:, :], otl[:, :], Silu, bias=bias2)
                            nc.sync.dma_start(
                                out.ap()[img, :, h0 * W : (h0 + RB) * W], stl[:, :]
                            )
                            if blk == NBLK - 2:
                                # drain target: first tile of the last pair,
                                # whose stt retires earliest
                                last_ps = ps
                            continue
                        g = blk % GS
                        if g == 0:
                            ot = otpool.tile([C, GS * NOUT], f16, name="ot", tag="ot")
                            st = stpool.tile([C, GS * NOUT], f16, name="st", tag="st")
                        # fused: ot = ps*scale2 + x; silu bias folds into ACT
                        nc.vector.scalar_tensor_tensor(
                            ot[:, g * NOUT : (g + 1) * NOUT].rearrange(
                                "p (h w) -> p h w", h=RB
                            ),
                            ps3[:, :, 0:W], scale2, xw, mult, add,
                        )
                        if g == GS - 1:
                            nc.scalar.activation(st[:, :], ot[:, :], Silu, bias=bias2)
                            nc.sync.dma_start(
                                out.ap()[img, :, (h0 - (GS - 1) * RB) * W : (h0 + RB) * W],
                                st[:, :],
                            )

            # trailing no-consumer matmul: the TileContext-exit DRAIN on the
            # PE queue otherwise swallows the last block's completion
            # semaphore flush. Writes over the last conv2 psum tile (WAR on
            # its stt read) so it fires right after the last epilogue read
            # instead of waiting for a free pool slot.
            nc.tensor.matmul(
                last_ps[:, 0:64], w2e_sb[:, :], w2e_sb[:, 0:64],
                start=True, stop=True,
            )

    nc.compile()
    return nc


def _quantize_ternary(w):
    """BitNet ternary quantization, matching the jax reference in fp32."""
    w = np.asarray(w, np.float32)
    scale = np.float32(max(np.float32(np.median(np.abs(w))), np.float32(1e-8)))
    tern = np.clip(np.round(w / scale), -1.0, 1.0).astype(np.float32)
    return tern, scale


def _pack_pairs(tern, pairs, f8dt):
    # lhsT layout [cin, pair, 2, cout] fp8
    return np.ascontiguousarray(
        np.stack(
            [
                np.stack([tern[:, :, ta[0] + 1, ta[1] + 1].T,
                          tern[:, :, tb[0] + 1, tb[1] + 1].T], axis=1)
                for (ta, tb) in pairs
            ],
            axis=1,
        ).astype(f8dt)
    )


def _host_prep(x, w1, b1, g1, be1, m1, v1, w2, b2, g2, be2, m2, v2):
    t1, s1 = _quantize_ternary(w1)
    t2, s2 = _quantize_ternary(w2)
    f8 = ml_dtypes.float8_e4m3
    wt1q = _pack_pairs(t1, PAIRS, f8)
    # corr pair over taps ((0,-1),(0,1)) with weights ternary/16
    tc16 = t1 / 16.0
    wt1c = np.ascontiguousarray(
        np.stack([tc16[:, :, 1, 0].T, tc16[:, :, 1, 2].T], axis=1).astype(f8)
    )
    wt1e = np.ascontiguousarray(t1[:, :, 1, 1].T.astype(np.float16))
    wt2q = _pack_pairs(t2, PAIRS, f8)
    wt2e = np.ascontiguousarray(t2[:, :, 1, 1].T.astype(np.float16))
    inv1 = (g1 / np.sqrt(v1 + BN_EPS)).astype(np.float32)
    inv2 = (g2 / np.sqrt(v2 + BN_EPS)).astype(np.float32)
    scale1 = s1 * inv1
    bias1 = b1 * inv1 + be1 - m1 * inv1
    scale2 = s2 * inv2
    bias2 = b2 * inv2 + be2 - m2 * inv2
    vecs = np.stack([scale1, bias1, scale2, bias2], axis=1).astype(np.float32)

    n = x.shape[0]
    x16 = np.zeros((n, C, L), dtype=np.float16)
    x8 = np.zeros((n, C, L), dtype=f8)
    dx8f = np.zeros((n, C, L), dtype=f8)
    x8v = x.astype(f8)
    dxv = np.clip(16.0 * (x - x8v.astype(np.float32)), -240, 240)
    for arr, val in ((x16, x), (x8, x8v), (dx8f, dxv)):
        a4 = arr[:, :, 0 : SW * LROWS].reshape(n, C, LROWS, SW)
        a4[:, :, 1 : 1 + H, 1 : 1 + W] = val
    # compact dx8: per block, the 115-elem span the corr pair reads
    dx8c = np.zeros((n, C, NBLK * CORRW2), dtype=f8)
    for b in range(NBLK):
        o = SW * (b * RB + 1)
        dx8c[:, :, b * CORRW2 : (b + 1) * CORRW2] = dx8f[:, :, o : o + CORRW2]
    return x16, x8, dx8c, wt1q, wt1c, wt1e, wt2e, wt2q, vecs


def kernel(
    x,
    w1,
    b1,
    bn1_gamma,
    bn1_beta,
    bn1_mean,
    bn1_var,
    w2,
    b2,
    bn2_gamma,
    bn2_beta,
    bn2_mean,
    bn2_var,
    _trace=False,
):
    from concourse.bass_utils import run_bass_kernel_spmd

    x = np.asarray(x, np.float32)
    w1, b1, w2, b2 = (np.asarray(a, np.float32) for a in (w1, b1, w2, b2))
    bn1_gamma, bn1_beta, bn1_mean, bn1_var = (
        np.asarray(a, np.float32) for a in (bn1_gamma, bn1_beta, bn1_mean, bn1_var)
    )
    bn2_gamma, bn2_beta, bn2_mean, bn2_var = (
        np.asarray(a, np.float32) for a in (bn2_gamma, bn2_beta, bn2_mean, bn2_var)
    )

    x16, x8, dx8c, wt1q, wt1c, wt1e, wt2e, wt2q, vecs = _host_prep(
        x, w1, b1, bn1_gamma, bn1_beta, bn1_mean, bn1_var,
        w2, b2, bn2_gamma, bn2_beta, bn2_mean, bn2_var,
    )

    if "nc" not in _CACHE:
        _CACHE["nc"] = _build_nc()
    nc = _CACHE["nc"]

    in_maps = [
        {
            "x16in": np.ascontiguousarray(x16[i * NPC : (i + 1) * NPC]),
            "x8in": np.ascontiguousarray(x8[i * NPC : (i + 1) * NPC]),
            "dx8cin": np.ascontiguousarray(dx8c[i * NPC : (i + 1) * NPC]),
            "wt1q": wt1q,
            "wt1c": wt1c,
            "wt1e": wt1e,
            "wt2e": wt2e,
            "wt2q": wt2q,
            "vecs": vecs,
        }
        for i in range(NCORES)
    ]
    res = run_bass_kernel_spmd(nc, in_maps, core_ids=list(range(NCORES)), trace=_trace)
    outs = [
        res.results[i]["out"].reshape(NPC, C, H, W).astype(np.float32)
        for i in range(NCORES)
    ]
    full = np.concatenate(outs, axis=0)
    if _trace:
        _CACHE["last_results"] = res
    return full
